# revision 1
# baseline (speedup 1.0000x reference)
"""Trainium2 Bass kernel for BodyStructureLoss.

Computes: mean over (B, J) of where(||kps[b,j,:]|| > 1.0, ||kps[b,j,:]||, 0)
for kps of shape [524288, 17, 3] float32.

Strategy (data-parallel over 8 NeuronCores):
  - Each core gets B/8 = 65536 batch rows = 3,342,336 contiguous floats,
    viewed as [128 partitions, 26112] (each partition row holds 8704
    complete (x,y,z) triplets).
  - Tiles of F columns stream in via DMA; squares run in-place (ACT, or DVE
    for a few tiles to balance engines); DVE sums the 3 squared components
    with two strided adds into a shared per-pair s tile; per tile PAIR one
    ACT sqrt and two DVE tensor_scalar+accumulate ops produce
    sum(max(d,1)) and count(s>1) columns.
  - Per core the [128, 2*n_pairs] accumulator tile is DMA'd out directly;
    the host sums all partials across cores and applies
    masked_sum = sum(max(d,1)) + count - B*J, then divides by B*J.
"""

import os

import numpy as np

# the NTFF trace path needs antenv.axon_hooks, which this client image lacks;
# force-disable so a stray BASS_TRACE=1 in the environment cannot break runs
os.environ["BASS_NEVER_TRACE"] = "1"

import concourse.bacc as bacc
import concourse.mybir as mybir
from concourse.bass_utils import run_bass_kernel_spmd
from concourse.tile import TileContext

B, J, D = 524288, 17, 3
HALF_BODY = 1.0  # threshold/2 with threshold=2.0
N_CORES = 8
B_SHARD = B // N_CORES  # 65536
P = 128
FLOATS_PER_CORE = B_SHARD * J * D  # 3342336
COLS = FLOATS_PER_CORE // P  # 26112 (divisible by 3: 26112 = 3*8704)

_DT = mybir.dt.float32

# default plan: pairs of tile column-counts; each pair shares one sqrt.
# small first pair ramps the compute pipeline early; small tail pairs
# shorten the post-DMA drain.
PLAN = [[456, 456]] + [[1728, 1536]] * 7 + [[768, 768], [360, 456]]
DVE_SQ = frozenset({9})  # tile indices squared on DVE instead of ACT
POOL_SQ = frozenset()  # tile indices squared on Pool (gpsimd)
# 1632-tiles whose square is split ACT (front 65%) / Pool (back 35%)
SPLIT_SQ = frozenset(range(2, 16))
# pairs that keep the count+max form (cheap 4x max on the tail spine);
# the host subtracts P*M2 per counted pair
STT_SKIP = frozenset({6, 7, 8, 9})


def build_nc(P=P, COLS=COLS, plan=None, dve_sq=DVE_SQ, pipelined=True, repeat=1, lag=2, flush_from=None, split_sq=SPLIT_SQ, pool_mode="stack", flush_before=None, pool_sq=POOL_SQ, count_pool=False, use_prio=False, wb_out=True, stt=True, add1_pool=frozenset(), stt_skip=STT_SKIP, xbufs=6, split_frac=0.65, adds_pool=frozenset({9}), s_bf16=False, split_frac_tail=None, tail_from=12):
    import bass_rust
    NameSet = bass_rust.InstructionNameOrderedSet
    if plan is None:
        plan = PLAN
    flat = [f for pair in plan for f in pair]
    assert sum(flat) == COLS
    assert all(f % 3 == 0 for f in flat)
    n_pairs = len(plan)
    M_MAX = max(sum(pair) for pair in plan) // 3

    nc = bacc.Bacc(
        "TRN2", target_bir_lowering=False, debug=False, num_devices=N_CORES
    )
    # stt pairs use one accumulator column; stt_skip pairs use two
    # (sum(max(d,1)) and count), requiring the host-side -M2*P correction
    col_of = {}
    off = 0
    for r in range(repeat):
        for pi0 in range(n_pairs):
            col_of[r * n_pairs + pi0] = off
            off += 2 if (not stt or pi0 in stt_skip) else 1
    acc_cols = off
    x = nc.dram_tensor("x", [P, COLS], _DT, kind="ExternalInput")
    out = nc.dram_tensor("out", [P, acc_cols], _DT, kind="ExternalOutput")

    with TileContext(nc, pool_alloc_mode=pool_mode) as tc:
        with (
            tc.tile_pool(name="xin", bufs=xbufs) as xpool,
            tc.tile_pool(name="small", bufs=4) as spool,
            tc.tile_pool(name="accp", bufs=1) as accpool,
        ):
            # stt: one accumulator column per pair, sum(d * (d > 1));
            # else two: sum(max(d,1)) and count(s>1)
            accs = accpool.tile([P, acc_cols], _DT)
            # shared scratch for tensor_scalar main outputs (only accum_out
            # is consumed); WAW chains are DVE-internal and in-order.
            # scr32 is only read by count pairs, so size it to their max M2.
            m32 = max([sum(plan[pi]) // 3 for pi in stt_skip] or [1]) if stt else M_MAX
            scr32 = accpool.tile([P, max(m32, 2)], _DT)
            scr16 = accpool.tile([P, M_MAX], mybir.dt.bfloat16)

            # dummy sqrt first: makes bacc's table pass load sqrt_and_others
            # (which also contains Square), avoiding a second ACT table load
            nc.vector.memset(scr32[:, :1], 1.0)
            nc.scalar.activation(
                out=scr32[:, :1],
                in_=scr32[:, :1],
                func=mybir.ActivationFunctionType.Sqrt,
            )

            acc_writers = []  # TS instructions writing accs (gate the out-DMA)
            if wb_out:
                wb_idx = accpool.tile([P, 1], mybir.dt.int32)
                nc.gpsimd.memset(wb_idx, 0)
                wb_sem = nc.alloc_semaphore("wb_dma")

            # scheduler priority: the tile scheduler pops the lowest
            # bass_priority READY instruction per engine; strict pair-order
            # priorities make a ready stage_b op always outrank later pairs'
            # squares on the same engine.
            def prio(ret, pi, off):
                if use_prio:
                    ret.ins.bass_priority = pi * 100 + off
                return ret

            # stage A (per tile): DMA -> square in place -> adds into s slice
            def stage_a(gi, pi, col0, F, s2, s_off):
                M = F // 3
                sz = str(F)
                nb = xbufs if F < 2448 else min(xbufs, 3)
                xb = {"bufs": nb} if nb != xbufs else {}
                xt = xpool.tile([P, F], _DT, tag="xt" + sz, **xb)
                prio(nc.sync.dma_start(out=xt, in_=x[:, col0 : col0 + F]), pi, 1)

                if gi in dve_sq:
                    r = nc.vector.tensor_tensor(
                        out=xt, in0=xt, in1=xt, op=mybir.AluOpType.mult
                    )
                elif gi in pool_sq:
                    r = nc.gpsimd.tensor_tensor(
                        out=xt, in0=xt, in1=xt, op=mybir.AluOpType.mult
                    )
                elif gi in split_sq:
                    # fractional engine split: ACT squares the front part,
                    # Pool (gpsimd) the back part of this tile. Chunks stay
                    # well under the pair cadence, so Pool cannot convoy.
                    f = split_frac if (split_frac_tail is None or gi < tail_from) else split_frac_tail
                    c = int(F * f) & ~3
                    prio(nc.scalar.activation(
                        out=xt[:, :c], in_=xt[:, :c],
                        func=mybir.ActivationFunctionType.Square,
                    ), pi, 2)
                    r = nc.gpsimd.tensor_tensor(
                        out=xt[:, c:], in0=xt[:, c:], in1=xt[:, c:],
                        op=mybir.AluOpType.mult,
                    )
                else:
                    r = nc.scalar.activation(
                        out=xt, in_=xt, func=mybir.ActivationFunctionType.Square
                    )
                prio(r, pi, 2)
                sq3 = xt.rearrange("p (m t) -> p m t", t=3)
                sl = s2[:, s_off : s_off + M]
                # adds_pool pairs run both adds on Pool: at the tail this
                # takes the s-computation off the (saturated) DVE chain
                eng1 = nc.gpsimd if pi in (add1_pool | adds_pool) else nc.vector
                eng2 = nc.gpsimd if pi in adds_pool else nc.vector
                prio(eng1.tensor_tensor(
                    out=sl, in0=sq3[:, :, 0], in1=sq3[:, :, 1], op=mybir.AluOpType.add
                ), pi, 3)
                prio(eng2.tensor_tensor(
                    out=sl, in0=sl, in1=sq3[:, :, 2], op=mybir.AluOpType.add
                ), pi, 4)

            # stage B (per pair): ACT sqrt -> DVE masked accumulate
            def stage_b(pi, s2, M2):
                sz = str(M2)
                c0 = col_of[pi]
                pi0 = pi % n_pairs
                use_stt = stt and pi0 not in stt_skip
                if not use_stt:
                    # count(s > 1): only depends on s, runs while ACT sqrts.
                    # With bf16 s the all-bf16 operands put this TS in DVE 4x
                    # mode. (TensorScalarPtr is DVE-only on core V3 —
                    # count_pool fails walrus codegen; sim experiments only.)
                    ceng = nc.gpsimd if count_pool else nc.vector
                    cscr = scr16 if s2.dtype == mybir.dt.bfloat16 else scr32
                    acc_writers.append(prio(ceng.tensor_scalar(
                        out=cscr[:, :M2],
                        in0=s2,
                        scalar1=float(HALF_BODY * HALF_BODY),
                        scalar2=None,
                        op0=mybir.AluOpType.is_gt,
                        op1=mybir.AluOpType.add,
                        accum_out=accs[:, c0 + 1 : c0 + 2],
                    ), pi, 5))
                db = {"bufs": 2} if M2 >= 2176 else {}
                d = spool.tile([P, M2], mybir.dt.bfloat16, tag="d" + sz, **db)
                prio(nc.scalar.activation(
                    out=d, in_=s2, func=mybir.ActivationFunctionType.Sqrt
                ), pi, 6)
                if use_stt:
                    # one fused DVE op: accum += sum((s > 1) * d). The mask
                    # comes from fp32 s (exact), so bf16 d contributes only
                    # symmetric per-element rounding noise that cancels.
                    acc_writers.append(prio(nc.vector.scalar_tensor_tensor(
                        out=scr16[:, :M2],
                        in0=s2,
                        scalar=float(HALF_BODY * HALF_BODY),
                        in1=d,
                        op0=mybir.AluOpType.is_gt,
                        op1=mybir.AluOpType.mult,
                        accum_out=accs[:, c0 : c0 + 1],
                    ), pi, 7))
                else:
                    # sum(max(d, 1)) = masked_sum + M2 - count per partition;
                    # bf16 all-operands makes this TS 4x mode (cheap on the
                    # tail spine). Host subtracts P*M2 for these pairs.
                    acc_writers.append(prio(nc.vector.tensor_scalar(
                        out=scr16[:, :M2],
                        in0=d,
                        scalar1=float(HALF_BODY),
                        scalar2=None,
                        op0=mybir.AluOpType.max,
                        op1=mybir.AluOpType.add,
                        accum_out=accs[:, c0 : c0 + 1],
                    ), pi, 7))

            # emit: stage_a per tile; stage_b lags `lag` pairs behind.
            # repeat>1 re-runs the whole pass (benchmarking only).
            from collections import deque

            pending = deque()  # (pair_idx, s2, M2)
            for r in range(repeat):
                col0 = 0
                gi = 0
                for pi0, pair in enumerate(plan):
                    pi = r * n_pairs + pi0
                    # emit pending stage_b BEFORE this pair's stage_a, so a
                    # ready sqrt is not queued behind a DMA-gated square
                    if flush_before is not None and pi0 >= flush_before:
                        while pending:
                            stage_b(*pending.popleft())
                    M2 = sum(pair) // 3
                    sb = {"bufs": 2} if M2 >= 2176 else {}
                    # bf16 s for count-style pairs: count/max both hit DVE 4x.
                    # Cost: the bf16 band just above s=1 is excluded from both
                    # count and the d>1 region (~1e-4 systematic, gate 2e-2).
                    sdt = (mybir.dt.bfloat16
                           if (s_bf16 and stt and pi0 in stt_skip) else _DT)
                    stag = ("sb" if sdt == mybir.dt.bfloat16 else "s") + str(M2)
                    s2 = spool.tile([P, M2], sdt, tag=stag, **sb)
                    s_off = 0
                    for F in pair:
                        stage_a(gi, pi, col0, F, s2, s_off)
                        col0 += F
                        s_off += F // 3
                        gi += 1
                    if pipelined:
                        pending.append((pi, s2, M2))
                        # from pair `flush_from` on, emit stage_b immediately
                        # so tail sqrts outrank later squares in the scheduler
                        eff_lag = 0 if (flush_from is not None and pi0 >= flush_from) else lag
                        while len(pending) > eff_lag:
                            stage_b(*pending.popleft())
                    else:
                        stage_b(pi, s2, M2)
            while pending:
                stage_b(*pending.popleft())

            if not wb_out:
                nc.sync.dma_start(out=out[:, :], in_=accs)
                wb_prep = None
            else:
                # out-DMA via pre-staged SWDGE descriptors: the prep only
                # writes descriptors (no HWDGE/dge-delay on the tail), the
                # trigger fires the tiny transfer after the last accum.
                in_view = accs[:, :].rearrange("p (a b w) -> p a b w", a=1, b=1)
                out_view = out[:, :].rearrange("p (a b w) -> a p b w", a=1, b=1)
                wb_prep = nc.gpsimd.kv_writeback(
                    out_view, in_view, wb_idx[:, :], prepare_only=True,
                    sem=wb_sem,
                )
                wb_trig = nc.gpsimd.trigger_dma(count=None)
                # defer the RAW edges on accs from the prep to the trigger
                # (what bass_rust's swdge_deferred_ins does for scatter_add;
                # kv_writeback is not in that table)
                acc_names = {w.ins.name for w in acc_writers}
                prep_sync = list(wb_prep.ins.sync_dependency_names())
                wb_prep.ins.set_sync_dependencies(
                    NameSet([d for d in prep_sync if d not in acc_names])
                )
                # no nosync edges either: the prep writes descriptors only,
                # so it carries no ordering against the accum writers at all
                wb_prep.ins.set_nosync_dependencies(NameSet(
                    [d for d in wb_prep.ins.nosync_dependency_names()
                     if d not in acc_names]
                ))
                wb_trig.ins.set_sync_dependencies(NameSet(
                    list(wb_trig.ins.sync_dependency_names())
                    + sorted(acc_names)
                ))

    nc.compile()

    if wb_prep is not None:
        # the cost model fires the prep's on_update[0] at trigger time, and
        # the teardown drain waits on the framework's DMASW queue sem; point
        # on_update[0] at that sem (scatter_add gets this wiring natively)
        dmasw = None
        for i in nc.all_instructions():
            if i.sync_info:
                for w in i.sync_info.on_wait:
                    if w.ant_name and w.ant_name.startswith("DMASW"):
                        dmasw = (w.id, w.ant_name)
        assert dmasw is not None, "no DMASW drain wait found"
        wb_prep.ins.sync_info.on_update[0] = mybir.SyncUpdate(
            sync_type="semaphore", id=dmasw[0], ant_name=dmasw[1],
            update_mode="sem-add-imm", update_value=16,
        )
    return nc


_nc_cache = None
last_results = None


def kernel(kps_world_pred: np.ndarray) -> np.ndarray:
    global _nc_cache, last_results
    x = np.ascontiguousarray(kps_world_pred, dtype=np.float32)
    assert x.shape == (B, J, D)

    shards = x.reshape(N_CORES, P, COLS)
    in_maps = [{"x": shards[c]} for c in range(N_CORES)]

    if _nc_cache is None:
        _nc_cache = build_nc()

    # the axon terminal occasionally reports a transient
    # NRT_EXEC_UNIT_UNRECOVERABLE left over from a previous run; it clears
    # after a short wait, so retry rather than fail the whole call
    import time

    res = None
    for attempt in range(3):
        try:
            res = run_bass_kernel_spmd(_nc_cache, in_maps, list(range(N_CORES)))
            break
        except Exception:
            if attempt == 2:
                raise
            time.sleep(15)
    last_results = res

    # stt pairs contribute sum(d * (d > 1)) directly; stt_skip pairs
    # contribute sum(max(d,1)) + count = masked_sum + P*M2 per core
    total = np.float64(0.0)
    for c in range(N_CORES):
        total += res.results[c]["out"].astype(np.float64).sum()
    skip_m2 = sum(sum(PLAN[pi]) // 3 for pi in STT_SKIP)
    total -= np.float64(N_CORES * P * skip_m2)
    return np.asarray(total / (B * J), dtype=np.float32)



# revision 6
# speedup vs baseline: 1.0131x; 1.0131x over previous
"""v2 Trainium2 Bass kernel for BodyStructureLoss (deinterleaved thirds).

Host deinterleaves each core's shard into [P, 3, 8704]: per partition row
[all x | all y | all z]. Per tile of M norm-columns:
  - one DMA brings [P, 3, M] (three contiguous M-runs per partition)
  - squares run in parallel per third: x->ACT, y->DVE, z->Pool (fp32->bf16)
  - s = x2+y2 (+z2) via two packed-bf16 DVE adds (2x mode)
  - count-TS (s>1, 4x) accumulates per tile
Tiles are grouped in PAIRS sharing one s2 buffer; per pair one ACT sqrt
(bf16) + one max-TS (max(d,1), 4x). Pairing keeps the ACT queue ahead of
the sqrt feedback loop (sqrt runs once per two DMA cadences).
Host: sum acc, subtract P*8704 per core (count+max identity), divide B*J.
"""

import os

import numpy as np

os.environ["BASS_NEVER_TRACE"] = "1"

import concourse.bacc as bacc
import concourse.mybir as mybir
from concourse.bass_utils import run_bass_kernel_spmd
from concourse.tile import TileContext

B, J, D = 524288, 17, 3
N_CORES = 8
P = 128
M_TOT = B // N_CORES * J // P  # 8704 norm columns per partition
_DT = mybir.dt.float32
_BF = mybir.dt.bfloat16

# pairs of tile sizes (norm-columns); a 1-element pair gets its own sqrt
PLAN = [[160, 160]] + [[576, 576]] * 6 + [[320, 320], [192, 192],
                                          [160, 160], [128]]
assert sum(m for pr in PLAN for m in pr) == M_TOT


def build_nc(plan=None, lag=1, xbufs=4, sq_eng=("act", "dve", "pool"),
             tail_dve_sq=0, last_flush=True, tail_pool=0, tail_edge=3,
             tail_eng=("act", "pool", "dve"), tail_eng_n=5,
             last2_eng=("act", "pool", "pool"), tail_lag_extra=0,
             sqrt_acc_last=1):
    import bass_rust
    from collections import Counter, deque
    NameSet = bass_rust.InstructionNameOrderedSet

    if plan is None:
        plan = PLAN
    flat = [m for pr in plan for m in pr]
    assert sum(flat) == M_TOT
    n_tiles = len(flat)
    n_pairs = len(plan)
    m_count = Counter(flat)
    m2_count = Counter(sum(pr) for pr in plan)

    nc = bacc.Bacc(
        "TRN2", target_bir_lowering=False, debug=False, num_devices=N_CORES
    )
    x = nc.dram_tensor("x", [P, 3, M_TOT], _DT, kind="ExternalInput")
    # acc columns: one count col per tile + one max col per pair
    acc_cols = n_tiles + n_pairs
    out = nc.dram_tensor("out", [P, acc_cols], _DT, kind="ExternalOutput")

    with TileContext(nc) as tc:
        with (
            tc.tile_pool(name="xin", bufs=xbufs) as xpool,
            tc.tile_pool(name="small", bufs=4) as spool,
            tc.tile_pool(name="accp", bufs=1) as accpool,
        ):
            eng_of = {"dve": nc.vector, "pool": nc.gpsimd}
            accs = accpool.tile([P, acc_cols], _DT)
            scr16 = accpool.tile([P, max(sum(pr) for pr in plan)], _BF)

            # dummy sqrt: preload the ACT table set containing Sqrt+Square
            nc.vector.memset(scr16[:, :1], 1.0)
            nc.scalar.activation(
                out=scr16[:, :1], in_=scr16[:, :1],
                func=mybir.ActivationFunctionType.Sqrt,
            )

            acc_writers = []
            wb_idx = accpool.tile([P, 1], mybir.dt.int32)
            nc.gpsimd.memset(wb_idx, 0)
            wb_sem = nc.alloc_semaphore("wb_dma")
            # scratch accum target: walrus rejects tensor_scalar without an
            # accum_out, so the elementwise max accumulates here (unused)
            dummy_acc = accpool.tile([P, 1], _DT)

            gi = [0]  # global tile index
            m_off = [0]
            sq_act_of = {}  # tile -> its ACT square handle (or None)

            def stage_a(ti, M, s2, s_off):
                sz = str(M)
                nb = min(xbufs, m_count[M])
                xt = xpool.tile([P, 3, M], _DT, tag="xt" + sz, bufs=nb)
                m0 = m_off[0]
                r = nc.sync.dma_start(out=xt, in_=x[:, :, m0 : m0 + M])
                NAME_MAP[r.ins.name] = ("dma", ti)
                m_off[0] += M
                x2 = spool.tile([P, 3, M], _BF, tag="x2" + sz,
                                bufs=min(4, m_count[M]))
                ndve = tail_dve_sq and ti >= n_tiles - tail_dve_sq
                npool = tail_pool and ti >= n_tiles - tail_pool
                engs = sq_eng
                if tail_eng and ti >= n_tiles - tail_eng_n:
                    engs = tail_eng
                if last2_eng and ti >= n_tiles - 2:
                    engs = last2_eng
                sq_act = None
                for k, ename in enumerate(engs):
                    if ndve:
                        ename = "dve"
                    elif npool and k > 0:
                        # tail: y,z squares on Pool to unload DVE's queue
                        ename = "pool"
                    if ename == "act":
                        r = nc.scalar.activation(
                            out=x2[:, k, :], in_=xt[:, k, :],
                            func=mybir.ActivationFunctionType.Square,
                        )
                        sq_act = r
                    else:
                        r = eng_of[ename].tensor_tensor(
                            out=x2[:, k, :], in0=xt[:, k, :], in1=xt[:, k, :],
                            op=mybir.AluOpType.mult,
                        )
                    NAME_MAP[r.ins.name] = ("sq_" + "xyz"[k], ti)
                sq_act_of[ti] = sq_act
                sl = s2[:, s_off : s_off + M]
                r = nc.vector.tensor_tensor(
                    out=sl, in0=x2[:, 0, :], in1=x2[:, 1, :],
                    op=mybir.AluOpType.add,
                )
                NAME_MAP[r.ins.name] = ("add1", ti)
                r = nc.vector.tensor_tensor(
                    out=sl, in0=sl, in1=x2[:, 2, :], op=mybir.AluOpType.add,
                )
                NAME_MAP[r.ins.name] = ("add2", ti)
                # count(s > 1) -> acc col ti (4x TS; independent of sqrt)
                r = nc.vector.tensor_scalar(
                    out=scr16[:, :M], in0=sl, scalar1=1.0, scalar2=None,
                    op0=mybir.AluOpType.is_gt, op1=mybir.AluOpType.add,
                    accum_out=accs[:, ti : ti + 1],
                )
                NAME_MAP[r.ins.name] = ("count", ti)
                acc_writers.append(r)

            def stage_b(pi, M2, s2, after=None):
                sz = str(M2)
                use_sqrt_acc = sqrt_acc_last and pi >= n_pairs - sqrt_acc_last
                d = spool.tile([P, M2], _BF, tag="d" + sz,
                               bufs=min(4, m2_count[M2]))
                if use_sqrt_acc:
                    # m = max(s,1) on DVE (4x, right after add2 in-queue),
                    # then ACT sqrt-with-accum: acc += sum(sqrt(m)) =
                    # sum(max(d,1)). Ends on ACT -> no post-sqrt DVE hop.
                    m = spool.tile([P, M2], _BF, tag="m" + sz,
                                   bufs=min(2, m2_count[M2]))
                    r = nc.vector.tensor_scalar(
                        out=m, in0=s2, scalar1=1.0, scalar2=None,
                        op0=mybir.AluOpType.max, op1=mybir.AluOpType.add,
                        accum_out=dummy_acc,
                    )
                    NAME_MAP[r.ins.name] = ("tsmax", pi)
                    r = nc.scalar.activation(
                        out=d, in_=m, func=mybir.ActivationFunctionType.Sqrt,
                        accum_out=accs[:, n_tiles + pi : n_tiles + pi + 1],
                    )
                    NAME_MAP[r.ins.name] = ("sqrt", pi)
                    if after is not None:
                        r.ins.set_nosync_dependencies(NameSet(
                            list(r.ins.nosync_dependency_names())
                            + [after.ins.name]
                        ))
                    acc_writers.append(r)
                    return
                r = nc.scalar.activation(
                    out=d, in_=s2, func=mybir.ActivationFunctionType.Sqrt,
                )
                NAME_MAP[r.ins.name] = ("sqrt", pi)
                if after is not None:
                    # scheduler-only edge: keep this sqrt BEHIND the newest
                    # tile's ACT square so squares stay DMA-anchored
                    r.ins.set_nosync_dependencies(NameSet(
                        list(r.ins.nosync_dependency_names())
                        + [after.ins.name]
                    ))
                # sum(max(d,1)) -> acc col n_tiles+pi (4x TS)
                r = nc.vector.tensor_scalar(
                    out=scr16[:, :M2], in0=d, scalar1=1.0, scalar2=None,
                    op0=mybir.AluOpType.max, op1=mybir.AluOpType.add,
                    accum_out=accs[:, n_tiles + pi : n_tiles + pi + 1],
                )
                NAME_MAP[r.ins.name] = ("max", pi)
                acc_writers.append(r)

            pending = deque()
            for pi, pr in enumerate(plan):
                M2 = sum(pr)
                s2 = spool.tile([P, M2], _BF, tag="s" + str(M2),
                                bufs=min(4, m2_count[M2]))
                s_off = 0
                for M in pr:
                    stage_a(gi[0], M, s2, s_off)
                    s_off += M
                    gi[0] += 1
                pending.append((pi, M2, s2))
                last_pair = pi == n_pairs - 1
                eff_lag = lag
                if pi >= n_pairs - tail_edge:
                    eff_lag = lag + tail_lag_extra
                if last_flush and last_pair:
                    eff_lag = 0
                while len(pending) > eff_lag:
                    if pi >= n_pairs - tail_edge:
                        # tail: order sqrts after the newest ACT square
                        aft = None
                        for tj in range(gi[0] - 1, -1, -1):
                            if sq_act_of.get(tj) is not None:
                                aft = sq_act_of[tj]
                                break
                    else:
                        aft = sq_act_of.get(gi[0] - len(pr))
                    stage_b(*pending.popleft(), after=aft)
            while pending:
                stage_b(*pending.popleft())

            # out-DMA via pre-staged SWDGE descriptors (prep early, trigger
            # after the final accumulate; RAW edges moved to the trigger)
            in_view = accs[:, :].rearrange("p (a b w) -> p a b w", a=1, b=1)
            out_view = out[:, :].rearrange("p (a b w) -> a p b w", a=1, b=1)
            wb_prep = nc.gpsimd.kv_writeback(
                out_view, in_view, wb_idx[:, :], prepare_only=True, sem=wb_sem,
            )
            wb_trig = nc.gpsimd.trigger_dma(count=None)
            acc_names = {w.ins.name for w in acc_writers}
            prep_sync = list(wb_prep.ins.sync_dependency_names())
            wb_prep.ins.set_sync_dependencies(
                NameSet([d for d in prep_sync if d not in acc_names])
            )
            wb_prep.ins.set_nosync_dependencies(NameSet(
                [d for d in wb_prep.ins.nosync_dependency_names()
                 if d not in acc_names]
            ))
            wb_trig.ins.set_sync_dependencies(NameSet(
                list(wb_trig.ins.sync_dependency_names()) + sorted(acc_names)
            ))

    nc.compile()

    # point the prep's on_update[0] at the DMASW drain sem (scatter_add-style
    # wiring; see v1 kernel for rationale)
    dmasw = None
    for i in nc.all_instructions():
        if i.sync_info:
            for w in i.sync_info.on_wait:
                if w.ant_name and w.ant_name.startswith("DMASW"):
                    dmasw = (w.id, w.ant_name)
    assert dmasw is not None, "no DMASW drain wait found"
    wb_prep.ins.sync_info.on_update[0] = mybir.SyncUpdate(
        sync_type="semaphore", id=dmasw[0], ant_name=dmasw[1],
        update_mode="sem-add-imm", update_value=16,
    )
    return nc


NAME_MAP = {}  # ins name -> (kind, index) for trace attribution

_nc_cache = None
last_results = None


def kernel(kps_world_pred: np.ndarray) -> np.ndarray:
    global _nc_cache, last_results
    x = np.ascontiguousarray(kps_world_pred, dtype=np.float32)
    assert x.shape == (B, J, D)

    # shard + deinterleave: [8, P, 8704 triplets, 3] -> [8, P, 3, 8704]
    v = np.ascontiguousarray(
        x.reshape(N_CORES, P, M_TOT, 3).transpose(0, 1, 3, 2)
    )
    in_maps = [{"x": v[c]} for c in range(N_CORES)]

    if _nc_cache is None:
        _nc_cache = build_nc()

    import time

    res = None
    for attempt in range(3):
        try:
            res = run_bass_kernel_spmd(_nc_cache, in_maps, list(range(N_CORES)))
            break
        except Exception:
            if attempt == 2:
                raise
            time.sleep(15)
    last_results = res

    # identity: sum(max(d,1)) + count(s>1) = masked_sum + P*M_TOT per core
    total = np.float64(0.0)
    for c in range(N_CORES):
        total += res.results[c]["out"].astype(np.float64).sum()
    total -= np.float64(N_CORES * P * M_TOT)
    return np.asarray(total / (B * J), dtype=np.float32)


# revision 7
# speedup vs baseline: 1.0152x; 1.0021x over previous
"""v2 Trainium2 Bass kernel for BodyStructureLoss (deinterleaved thirds).

Host deinterleaves each core's shard into [P, 3, 8704]: per partition row
[all x | all y | all z]. Per tile of M norm-columns:
  - one DMA brings [P, 3, M] (three contiguous M-runs per partition)
  - squares run in parallel per third: x->ACT, y->DVE, z->Pool (fp32->bf16)
  - s = x2+y2 (+z2) via two packed-bf16 DVE adds (2x mode)
  - count-TS (s>1, 4x) accumulates per tile
Tiles are grouped in PAIRS sharing one s2 buffer; per pair one ACT sqrt
(bf16) + one max-TS (max(d,1), 4x). Pairing keeps the ACT queue ahead of
the sqrt feedback loop (sqrt runs once per two DMA cadences).
Host: sum acc, subtract P*8704 per core (count+max identity), divide B*J.
"""

import os

import numpy as np

os.environ["BASS_NEVER_TRACE"] = "1"

import concourse.bacc as bacc
import concourse.mybir as mybir
from concourse.bass_utils import run_bass_kernel_spmd
from concourse.tile import TileContext

B, J, D = 524288, 17, 3
N_CORES = 8
P = 128
M_TOT = B // N_CORES * J // P  # 8704 norm columns per partition
_DT = mybir.dt.float32
_BF = mybir.dt.bfloat16

# pairs of tile sizes (norm-columns); a 1-element pair gets its own sqrt
PLAN = [[160, 160]] + [[576, 576]] * 6 + [[336, 336], [176, 176],
                                          [160, 160], [128]]
assert sum(m for pr in PLAN for m in pr) == M_TOT


def build_nc(plan=None, lag=1, xbufs=4, sq_eng=("act", "dve", "pool"),
             tail_dve_sq=0, last_flush=True, tail_pool=0, tail_edge=3,
             tail_eng=("act", "pool", "dve"), tail_eng_n=5,
             last2_eng=("act", "pool", "pool"), tail_lag_extra=0,
             sqrt_acc_last=1):
    import bass_rust
    from collections import Counter, deque
    NameSet = bass_rust.InstructionNameOrderedSet

    if plan is None:
        plan = PLAN
    flat = [m for pr in plan for m in pr]
    assert sum(flat) == M_TOT
    n_tiles = len(flat)
    n_pairs = len(plan)
    m_count = Counter(flat)
    m2_count = Counter(sum(pr) for pr in plan)

    nc = bacc.Bacc(
        "TRN2", target_bir_lowering=False, debug=False, num_devices=N_CORES
    )
    x = nc.dram_tensor("x", [P, 3, M_TOT], _DT, kind="ExternalInput")
    # acc columns: one count col per tile + one max col per pair
    acc_cols = n_tiles + n_pairs
    out = nc.dram_tensor("out", [P, acc_cols], _DT, kind="ExternalOutput")

    with TileContext(nc) as tc:
        with (
            tc.tile_pool(name="xin", bufs=xbufs) as xpool,
            tc.tile_pool(name="small", bufs=4) as spool,
            tc.tile_pool(name="accp", bufs=1) as accpool,
        ):
            eng_of = {"dve": nc.vector, "pool": nc.gpsimd}
            accs = accpool.tile([P, acc_cols], _DT)
            scr16 = accpool.tile([P, max(sum(pr) for pr in plan)], _BF)

            # dummy sqrt: preload the ACT table set containing Sqrt+Square
            nc.vector.memset(scr16[:, :1], 1.0)
            nc.scalar.activation(
                out=scr16[:, :1], in_=scr16[:, :1],
                func=mybir.ActivationFunctionType.Sqrt,
            )

            acc_writers = []
            wb_idx = accpool.tile([P, 1], mybir.dt.int32)
            nc.gpsimd.memset(wb_idx, 0)
            wb_sem = nc.alloc_semaphore("wb_dma")
            # scratch accum target: walrus rejects tensor_scalar without an
            # accum_out, so the elementwise max accumulates here (unused)
            dummy_acc = accpool.tile([P, 1], _DT)

            gi = [0]  # global tile index
            m_off = [0]
            sq_act_of = {}  # tile -> its ACT square handle (or None)

            def stage_a(ti, M, s2, s_off):
                sz = str(M)
                nb = min(xbufs, m_count[M])
                xt = xpool.tile([P, 3, M], _DT, tag="xt" + sz, bufs=nb)
                m0 = m_off[0]
                r = nc.sync.dma_start(out=xt, in_=x[:, :, m0 : m0 + M])
                NAME_MAP[r.ins.name] = ("dma", ti)
                m_off[0] += M
                x2 = spool.tile([P, 3, M], _BF, tag="x2" + sz,
                                bufs=min(4, m_count[M]))
                ndve = tail_dve_sq and ti >= n_tiles - tail_dve_sq
                npool = tail_pool and ti >= n_tiles - tail_pool
                engs = sq_eng
                if tail_eng and ti >= n_tiles - tail_eng_n:
                    engs = tail_eng
                if last2_eng and ti >= n_tiles - 2:
                    engs = last2_eng
                sq_act = None
                for k, ename in enumerate(engs):
                    if ndve:
                        ename = "dve"
                    elif npool and k > 0:
                        # tail: y,z squares on Pool to unload DVE's queue
                        ename = "pool"
                    if ename == "act":
                        r = nc.scalar.activation(
                            out=x2[:, k, :], in_=xt[:, k, :],
                            func=mybir.ActivationFunctionType.Square,
                        )
                        sq_act = r
                    else:
                        r = eng_of[ename].tensor_tensor(
                            out=x2[:, k, :], in0=xt[:, k, :], in1=xt[:, k, :],
                            op=mybir.AluOpType.mult,
                        )
                    NAME_MAP[r.ins.name] = ("sq_" + "xyz"[k], ti)
                sq_act_of[ti] = sq_act
                sl = s2[:, s_off : s_off + M]
                r = nc.vector.tensor_tensor(
                    out=sl, in0=x2[:, 0, :], in1=x2[:, 1, :],
                    op=mybir.AluOpType.add,
                )
                NAME_MAP[r.ins.name] = ("add1", ti)
                r = nc.vector.tensor_tensor(
                    out=sl, in0=sl, in1=x2[:, 2, :], op=mybir.AluOpType.add,
                )
                NAME_MAP[r.ins.name] = ("add2", ti)
                # count(s > 1) -> acc col ti (4x TS; independent of sqrt)
                r = nc.vector.tensor_scalar(
                    out=scr16[:, :M], in0=sl, scalar1=1.0, scalar2=None,
                    op0=mybir.AluOpType.is_gt, op1=mybir.AluOpType.add,
                    accum_out=accs[:, ti : ti + 1],
                )
                NAME_MAP[r.ins.name] = ("count", ti)
                acc_writers.append(r)

            def stage_b(pi, M2, s2, after=None):
                sz = str(M2)
                use_sqrt_acc = sqrt_acc_last and pi >= n_pairs - sqrt_acc_last
                d = spool.tile([P, M2], _BF, tag="d" + sz,
                               bufs=min(4, m2_count[M2]))
                if use_sqrt_acc:
                    # m = max(s,1) on DVE (4x, right after add2 in-queue),
                    # then ACT sqrt-with-accum: acc += sum(sqrt(m)) =
                    # sum(max(d,1)). Ends on ACT -> no post-sqrt DVE hop.
                    m = spool.tile([P, M2], _BF, tag="m" + sz,
                                   bufs=min(2, m2_count[M2]))
                    r = nc.vector.tensor_scalar(
                        out=m, in0=s2, scalar1=1.0, scalar2=None,
                        op0=mybir.AluOpType.max, op1=mybir.AluOpType.add,
                        accum_out=dummy_acc,
                    )
                    NAME_MAP[r.ins.name] = ("tsmax", pi)
                    r = nc.scalar.activation(
                        out=d, in_=m, func=mybir.ActivationFunctionType.Sqrt,
                        accum_out=accs[:, n_tiles + pi : n_tiles + pi + 1],
                    )
                    NAME_MAP[r.ins.name] = ("sqrt", pi)
                    if after is not None:
                        r.ins.set_nosync_dependencies(NameSet(
                            list(r.ins.nosync_dependency_names())
                            + [after.ins.name]
                        ))
                    acc_writers.append(r)
                    return
                r = nc.scalar.activation(
                    out=d, in_=s2, func=mybir.ActivationFunctionType.Sqrt,
                )
                NAME_MAP[r.ins.name] = ("sqrt", pi)
                if after is not None:
                    # scheduler-only edge: keep this sqrt BEHIND the newest
                    # tile's ACT square so squares stay DMA-anchored
                    r.ins.set_nosync_dependencies(NameSet(
                        list(r.ins.nosync_dependency_names())
                        + [after.ins.name]
                    ))
                # sum(max(d,1)) -> acc col n_tiles+pi (4x TS)
                r = nc.vector.tensor_scalar(
                    out=scr16[:, :M2], in0=d, scalar1=1.0, scalar2=None,
                    op0=mybir.AluOpType.max, op1=mybir.AluOpType.add,
                    accum_out=accs[:, n_tiles + pi : n_tiles + pi + 1],
                )
                NAME_MAP[r.ins.name] = ("max", pi)
                acc_writers.append(r)

            pending = deque()
            for pi, pr in enumerate(plan):
                M2 = sum(pr)
                s2 = spool.tile([P, M2], _BF, tag="s" + str(M2),
                                bufs=min(4, m2_count[M2]))
                s_off = 0
                for M in pr:
                    stage_a(gi[0], M, s2, s_off)
                    s_off += M
                    gi[0] += 1
                pending.append((pi, M2, s2))
                last_pair = pi == n_pairs - 1
                eff_lag = lag
                if pi >= n_pairs - tail_edge:
                    eff_lag = lag + tail_lag_extra
                if last_flush and last_pair:
                    eff_lag = 0
                while len(pending) > eff_lag:
                    if pi >= n_pairs - tail_edge:
                        # tail: order sqrts after the newest ACT square
                        aft = None
                        for tj in range(gi[0] - 1, -1, -1):
                            if sq_act_of.get(tj) is not None:
                                aft = sq_act_of[tj]
                                break
                    else:
                        aft = sq_act_of.get(gi[0] - len(pr))
                    stage_b(*pending.popleft(), after=aft)
            while pending:
                stage_b(*pending.popleft())

            # out-DMA via pre-staged SWDGE descriptors (prep early, trigger
            # after the final accumulate; RAW edges moved to the trigger)
            in_view = accs[:, :].rearrange("p (a b w) -> p a b w", a=1, b=1)
            out_view = out[:, :].rearrange("p (a b w) -> a p b w", a=1, b=1)
            wb_prep = nc.gpsimd.kv_writeback(
                out_view, in_view, wb_idx[:, :], prepare_only=True, sem=wb_sem,
            )
            wb_trig = nc.gpsimd.trigger_dma(count=None)
            acc_names = {w.ins.name for w in acc_writers}
            prep_sync = list(wb_prep.ins.sync_dependency_names())
            wb_prep.ins.set_sync_dependencies(
                NameSet([d for d in prep_sync if d not in acc_names])
            )
            wb_prep.ins.set_nosync_dependencies(NameSet(
                [d for d in wb_prep.ins.nosync_dependency_names()
                 if d not in acc_names]
            ))
            wb_trig.ins.set_sync_dependencies(NameSet(
                list(wb_trig.ins.sync_dependency_names()) + sorted(acc_names)
            ))

    nc.compile()

    # point the prep's on_update[0] at the DMASW drain sem (scatter_add-style
    # wiring; see v1 kernel for rationale)
    dmasw = None
    for i in nc.all_instructions():
        if i.sync_info:
            for w in i.sync_info.on_wait:
                if w.ant_name and w.ant_name.startswith("DMASW"):
                    dmasw = (w.id, w.ant_name)
    assert dmasw is not None, "no DMASW drain wait found"
    wb_prep.ins.sync_info.on_update[0] = mybir.SyncUpdate(
        sync_type="semaphore", id=dmasw[0], ant_name=dmasw[1],
        update_mode="sem-add-imm", update_value=16,
    )
    return nc


NAME_MAP = {}  # ins name -> (kind, index) for trace attribution

_nc_cache = None
last_results = None


def kernel(kps_world_pred: np.ndarray) -> np.ndarray:
    global _nc_cache, last_results
    x = np.ascontiguousarray(kps_world_pred, dtype=np.float32)
    assert x.shape == (B, J, D)

    # shard + deinterleave: [8, P, 8704 triplets, 3] -> [8, P, 3, 8704]
    v = np.ascontiguousarray(
        x.reshape(N_CORES, P, M_TOT, 3).transpose(0, 1, 3, 2)
    )
    in_maps = [{"x": v[c]} for c in range(N_CORES)]

    if _nc_cache is None:
        _nc_cache = build_nc()

    import time

    res = None
    for attempt in range(3):
        try:
            res = run_bass_kernel_spmd(_nc_cache, in_maps, list(range(N_CORES)))
            break
        except Exception:
            if attempt == 2:
                raise
            time.sleep(15)
    last_results = res

    # identity: sum(max(d,1)) + count(s>1) = masked_sum + P*M_TOT per core
    total = np.float64(0.0)
    for c in range(N_CORES):
        total += res.results[c]["out"].astype(np.float64).sum()
    total -= np.float64(N_CORES * P * M_TOT)
    return np.asarray(total / (B * J), dtype=np.float32)


# revision 8
# speedup vs baseline: 1.0208x; 1.0055x over previous
"""v2 Trainium2 Bass kernel for BodyStructureLoss (deinterleaved thirds).

Host deinterleaves each core's shard into [P, 3, 8704]: per partition row
[all x | all y | all z]. Per tile of M norm-columns:
  - one DMA brings [P, 3, M] (three contiguous M-runs per partition)
  - squares run in parallel per third: x->ACT, y->DVE, z->Pool (fp32->bf16)
  - s = x2+y2 (+z2) via two packed-bf16 DVE adds (2x mode)
  - count-TS (s>1, 4x) accumulates per tile
Tiles are grouped in PAIRS sharing one s2 buffer; per pair one ACT sqrt
(bf16) + one max-TS (max(d,1), 4x). Pairing keeps the ACT queue ahead of
the sqrt feedback loop (sqrt runs once per two DMA cadences).
Host: sum acc, subtract P*8704 per core (count+max identity), divide B*J.
"""

import os

import numpy as np

os.environ["BASS_NEVER_TRACE"] = "1"

import concourse.bacc as bacc
import concourse.mybir as mybir
from concourse.bass_utils import run_bass_kernel_spmd
from concourse.tile import TileContext

B, J, D = 524288, 17, 3
N_CORES = 8
P = 128
M_TOT = B // N_CORES * J // P  # 8704 norm columns per partition
_DT = mybir.dt.float32
_BF = mybir.dt.bfloat16

# pairs of tile sizes (norm-columns); a 1-element pair gets its own sqrt
PLAN = [[160, 160]] + [[576, 576]] * 6 + [[336, 336], [176, 176],
                                          [160, 160], [128]]
assert sum(m for pr in PLAN for m in pr) == M_TOT


def build_nc(plan=None, lag=1, xbufs=4, sq_eng=("act", "dve", "pool"),
             tail_dve_sq=0, last_flush=True, tail_pool=0, tail_edge=3,
             tail_eng=("act", "pool", "dve"), tail_eng_n=7,
             last2_eng=("act", "pool", "pool"), tail_lag_extra=0,
             sqrt_acc_last=1, pin_max_tail=0, endgame_edges=True):
    import bass_rust
    from collections import Counter, deque
    NameSet = bass_rust.InstructionNameOrderedSet

    if plan is None:
        plan = PLAN
    flat = [m for pr in plan for m in pr]
    assert sum(flat) == M_TOT
    n_tiles = len(flat)
    n_pairs = len(plan)
    m_count = Counter(flat)
    m2_count = Counter(sum(pr) for pr in plan)

    nc = bacc.Bacc(
        "TRN2", target_bir_lowering=False, debug=False, num_devices=N_CORES
    )
    x = nc.dram_tensor("x", [P, 3, M_TOT], _DT, kind="ExternalInput")
    # acc columns: one count col per tile + one max col per pair
    acc_cols = n_tiles + n_pairs
    out = nc.dram_tensor("out", [P, acc_cols], _DT, kind="ExternalOutput")

    with TileContext(nc) as tc:
        with (
            tc.tile_pool(name="xin", bufs=xbufs) as xpool,
            tc.tile_pool(name="small", bufs=4) as spool,
            tc.tile_pool(name="accp", bufs=1) as accpool,
        ):
            eng_of = {"dve": nc.vector, "pool": nc.gpsimd}
            accs = accpool.tile([P, acc_cols], _DT)
            scr16 = accpool.tile([P, max(sum(pr) for pr in plan)], _BF)

            # dummy sqrt: preload the ACT table set containing Sqrt+Square
            nc.vector.memset(scr16[:, :1], 1.0)
            nc.scalar.activation(
                out=scr16[:, :1], in_=scr16[:, :1],
                func=mybir.ActivationFunctionType.Sqrt,
            )

            acc_writers = []
            wb_idx = accpool.tile([P, 1], mybir.dt.int32)
            nc.gpsimd.memset(wb_idx, 0)
            wb_sem = nc.alloc_semaphore("wb_dma")
            # scratch accum target: walrus rejects tensor_scalar without an
            # accum_out, so the elementwise max accumulates here (unused)
            dummy_acc = accpool.tile([P, 1], _DT)

            gi = [0]  # global tile index
            m_off = [0]
            sq_act_of = {}  # tile -> its ACT square handle (or None)
            last_max = [None]  # most recent max/tsmax handle (DVE)
            count_of = {}  # tile -> count handle
            tsmax_of = {}  # pair -> tsmax handle (sqrt_acc path)
            max_of = {}  # pair -> max handle (regular path)

            def stage_a(ti, M, s2, s_off):
                sz = str(M)
                nb = min(xbufs, m_count[M])
                xt = xpool.tile([P, 3, M], _DT, tag="xt" + sz, bufs=nb)
                m0 = m_off[0]
                r = nc.sync.dma_start(out=xt, in_=x[:, :, m0 : m0 + M])
                NAME_MAP[r.ins.name] = ("dma", ti)
                m_off[0] += M
                x2 = spool.tile([P, 3, M], _BF, tag="x2" + sz,
                                bufs=min(4, m_count[M]))
                ndve = tail_dve_sq and ti >= n_tiles - tail_dve_sq
                npool = tail_pool and ti >= n_tiles - tail_pool
                engs = sq_eng
                if tail_eng and ti >= n_tiles - tail_eng_n:
                    engs = tail_eng
                if last2_eng and ti >= n_tiles - 2:
                    engs = last2_eng
                sq_act = None
                for k, ename in enumerate(engs):
                    if ndve:
                        ename = "dve"
                    elif npool and k > 0:
                        # tail: y,z squares on Pool to unload DVE's queue
                        ename = "pool"
                    if ename == "act":
                        r = nc.scalar.activation(
                            out=x2[:, k, :], in_=xt[:, k, :],
                            func=mybir.ActivationFunctionType.Square,
                        )
                        sq_act = r
                    else:
                        r = eng_of[ename].tensor_tensor(
                            out=x2[:, k, :], in0=xt[:, k, :], in1=xt[:, k, :],
                            op=mybir.AluOpType.mult,
                        )
                    NAME_MAP[r.ins.name] = ("sq_" + "xyz"[k], ti)
                sq_act_of[ti] = sq_act
                sl = s2[:, s_off : s_off + M]
                r = nc.vector.tensor_tensor(
                    out=sl, in0=x2[:, 0, :], in1=x2[:, 1, :],
                    op=mybir.AluOpType.add,
                )
                NAME_MAP[r.ins.name] = ("add1", ti)
                if (pin_max_tail and ti >= n_tiles - pin_max_tail
                        and last_max[0] is not None):
                    # keep earlier pairs' max ops AHEAD of the tail adds on
                    # DVE so they don't pollute the endgame queue
                    r.ins.set_nosync_dependencies(NameSet(
                        list(r.ins.nosync_dependency_names())
                        + [last_max[0].ins.name]
                    ))
                r = nc.vector.tensor_tensor(
                    out=sl, in0=sl, in1=x2[:, 2, :], op=mybir.AluOpType.add,
                )
                NAME_MAP[r.ins.name] = ("add2", ti)
                # count(s > 1) -> acc col ti (4x TS; independent of sqrt)
                cscr = spool.tile([P, M], _BF, tag="c" + sz,
                                  bufs=min(2, m_count[M]))
                r = nc.vector.tensor_scalar(
                    out=cscr, in0=sl, scalar1=1.0, scalar2=None,
                    op0=mybir.AluOpType.is_gt, op1=mybir.AluOpType.add,
                    accum_out=accs[:, ti : ti + 1],
                )
                NAME_MAP[r.ins.name] = ("count", ti)
                count_of[ti] = r
                acc_writers.append(r)

            def stage_b(pi, M2, s2, after=None):
                sz = str(M2)
                use_sqrt_acc = sqrt_acc_last and pi >= n_pairs - sqrt_acc_last
                d = spool.tile([P, M2], _BF, tag="d" + sz,
                               bufs=min(4, m2_count[M2]))
                if use_sqrt_acc:
                    # m = max(s,1) on DVE (4x, right after add2 in-queue),
                    # then ACT sqrt-with-accum: acc += sum(sqrt(m)) =
                    # sum(max(d,1)). Ends on ACT -> no post-sqrt DVE hop.
                    m = spool.tile([P, M2], _BF, tag="m" + sz,
                                   bufs=min(2, m2_count[M2]))
                    r = nc.vector.tensor_scalar(
                        out=m, in0=s2, scalar1=1.0, scalar2=None,
                        op0=mybir.AluOpType.max, op1=mybir.AluOpType.add,
                        accum_out=dummy_acc,
                    )
                    NAME_MAP[r.ins.name] = ("tsmax", pi)
                    tsmax_of[pi] = r
                    r = nc.scalar.activation(
                        out=d, in_=m, func=mybir.ActivationFunctionType.Sqrt,
                        accum_out=accs[:, n_tiles + pi : n_tiles + pi + 1],
                    )
                    NAME_MAP[r.ins.name] = ("sqrt", pi)
                    if after is not None:
                        r.ins.set_nosync_dependencies(NameSet(
                            list(r.ins.nosync_dependency_names())
                            + [after.ins.name]
                        ))
                    acc_writers.append(r)
                    return
                r = nc.scalar.activation(
                    out=d, in_=s2, func=mybir.ActivationFunctionType.Sqrt,
                )
                NAME_MAP[r.ins.name] = ("sqrt", pi)
                if after is not None:
                    # scheduler-only edge: keep this sqrt BEHIND the newest
                    # tile's ACT square so squares stay DMA-anchored
                    r.ins.set_nosync_dependencies(NameSet(
                        list(r.ins.nosync_dependency_names())
                        + [after.ins.name]
                    ))
                # sum(max(d,1)) -> acc col n_tiles+pi (4x TS)
                mscr = spool.tile([P, M2], _BF, tag="mx" + sz,
                                  bufs=min(2, m2_count[M2]))
                r = nc.vector.tensor_scalar(
                    out=mscr, in0=d, scalar1=1.0, scalar2=None,
                    op0=mybir.AluOpType.max, op1=mybir.AluOpType.add,
                    accum_out=accs[:, n_tiles + pi : n_tiles + pi + 1],
                )
                NAME_MAP[r.ins.name] = ("max", pi)
                max_of[pi] = r
                acc_writers.append(r)
                last_max[0] = r

            pending = deque()
            for pi, pr in enumerate(plan):
                M2 = sum(pr)
                s2 = spool.tile([P, M2], _BF, tag="s" + str(M2),
                                bufs=min(4, m2_count[M2]))
                s_off = 0
                for M in pr:
                    stage_a(gi[0], M, s2, s_off)
                    s_off += M
                    gi[0] += 1
                pending.append((pi, M2, s2))
                last_pair = pi == n_pairs - 1
                eff_lag = lag
                if pi >= n_pairs - tail_edge:
                    eff_lag = lag + tail_lag_extra
                if last_flush and last_pair:
                    eff_lag = 0
                while len(pending) > eff_lag:
                    if pi >= n_pairs - tail_edge:
                        # tail: order sqrts after the newest ACT square
                        aft = None
                        for tj in range(gi[0] - 1, -1, -1):
                            if sq_act_of.get(tj) is not None:
                                aft = sq_act_of[tj]
                                break
                    else:
                        aft = sq_act_of.get(gi[0] - len(pr))
                    stage_b(*pending.popleft(), after=aft)
            while pending:
                stage_b(*pending.popleft())

            if endgame_edges:
                def add_nosync(ins_r, after_r):
                    nm = after_r.ins.name
                    if (nm in ins_r.ins.sync_dependency_names()
                            or nm in ins_r.ins.nosync_dependency_names()):
                        return
                    ins_r.ins.set_nosync_dependencies(NameSet(
                        list(ins_r.ins.nosync_dependency_names()) + [nm]
                    ))
                # keep late mid-pair maxes out of the endgame DVE chain:
                # order them after the last tile's count
                lc = count_of.get(n_tiles - 1)
                if lc is not None:
                    for pj in range(max(0, n_pairs - 4), n_pairs):
                        if pj in max_of:
                            add_nosync(max_of[pj], lc)
                # let the last count overlap the final sqrt: order it after
                # the last pair's tsmax on DVE
                tm = tsmax_of.get(n_pairs - 1)
                if tm is not None and lc is not None:
                    add_nosync(lc, tm)

            # out-DMA via pre-staged SWDGE descriptors (prep early, trigger
            # after the final accumulate; RAW edges moved to the trigger)
            in_view = accs[:, :].rearrange("p (a b w) -> p a b w", a=1, b=1)
            out_view = out[:, :].rearrange("p (a b w) -> a p b w", a=1, b=1)
            wb_prep = nc.gpsimd.kv_writeback(
                out_view, in_view, wb_idx[:, :], prepare_only=True, sem=wb_sem,
            )
            wb_trig = nc.gpsimd.trigger_dma(count=None)
            acc_names = {w.ins.name for w in acc_writers}
            prep_sync = list(wb_prep.ins.sync_dependency_names())
            wb_prep.ins.set_sync_dependencies(
                NameSet([d for d in prep_sync if d not in acc_names])
            )
            wb_prep.ins.set_nosync_dependencies(NameSet(
                [d for d in wb_prep.ins.nosync_dependency_names()
                 if d not in acc_names]
            ))
            wb_trig.ins.set_sync_dependencies(NameSet(
                list(wb_trig.ins.sync_dependency_names()) + sorted(acc_names)
            ))

    nc.compile()

    # point the prep's on_update[0] at the DMASW drain sem (scatter_add-style
    # wiring; see v1 kernel for rationale)
    dmasw = None
    for i in nc.all_instructions():
        if i.sync_info:
            for w in i.sync_info.on_wait:
                if w.ant_name and w.ant_name.startswith("DMASW"):
                    dmasw = (w.id, w.ant_name)
    assert dmasw is not None, "no DMASW drain wait found"
    wb_prep.ins.sync_info.on_update[0] = mybir.SyncUpdate(
        sync_type="semaphore", id=dmasw[0], ant_name=dmasw[1],
        update_mode="sem-add-imm", update_value=16,
    )
    return nc


NAME_MAP = {}  # ins name -> (kind, index) for trace attribution

_nc_cache = None
last_results = None


def kernel(kps_world_pred: np.ndarray) -> np.ndarray:
    global _nc_cache, last_results
    x = np.ascontiguousarray(kps_world_pred, dtype=np.float32)
    assert x.shape == (B, J, D)

    # shard + deinterleave: [8, P, 8704 triplets, 3] -> [8, P, 3, 8704]
    v = np.ascontiguousarray(
        x.reshape(N_CORES, P, M_TOT, 3).transpose(0, 1, 3, 2)
    )
    in_maps = [{"x": v[c]} for c in range(N_CORES)]

    if _nc_cache is None:
        _nc_cache = build_nc()

    import time

    res = None
    for attempt in range(3):
        try:
            res = run_bass_kernel_spmd(_nc_cache, in_maps, list(range(N_CORES)))
            break
        except Exception:
            if attempt == 2:
                raise
            time.sleep(15)
    last_results = res

    # identity: sum(max(d,1)) + count(s>1) = masked_sum + P*M_TOT per core
    total = np.float64(0.0)
    for c in range(N_CORES):
        total += res.results[c]["out"].astype(np.float64).sum()
    total -= np.float64(N_CORES * P * M_TOT)
    return np.asarray(total / (B * J), dtype=np.float32)
